# revision 8
# baseline (speedup 1.0000x reference)
"""CapsLayer kernel v6: j-sharded, W-stationary bf16 contraction.

Math: the reference's routing loop is dead (softmax over a size-1 axis is
identically 1), so the output is
    s[b, j, l] = sum_{i,k} W[i, j, l, k] * inputs[b, i, k]
    vj = squash(s, axis=l)  ->  [B, 1, NUM_CAPS, DIM_CAPS]

Sharding: W splits over NUM_CAPS j (4 capsules / 128 output columns per
core); inputs are replicated.  No collectives (an 8-core ncfw
ReduceScatter measures ~42 us of fixed latency).

Precision: x/W stream in bf16 (10.5 MB/core; the fp32 stream was already
at the ~360 B/ns DMA bus limit, so bytes are the whole game), PSUM
accumulation and the squash run in fp32.  Rel err ~2.6e-3 vs 2e-2 gate.

PE orientation (v6): W[128i, 128(j,l)] is the STATIONARY operand — full
128 columns enables the compiler's fast-weight-load (2 bf16/cycle over 4
XBUSes) — and x[128i, 32b] streams as moving data.  One accumulation
chain of 256 matmuls into a single PSUM bank s_T[(j,l), b]; v4's 4-way
column tiling (x stationary, W moving) pushed all of W through the
1-col/cycle moving bus and made the PE a co-bottleneck with the DMA, and
needed an extra identity matmul to merge the 4 chains.  The transposed
result costs 4 DVE 32x32 stream-transposes in the epilogue, cheaper than
the merge matmul + PSUM copy it replaces.

DMA: 17 transfers — one per 128-row i-tile, with the last tile split in
half (k 0-7 / k 8-15) so the final PE chunk starts half a tile sooner.
Rows pack [x(k0-7) W(k0-7) | x(k8-15) W(k8-15)] so each half is one
contiguous 2560 B descriptor per partition.

Raw Bass: this walrus build rejects instructions carrying 2+ sem waits, so
all sync is standalone wait_ge ops.  DVE/ACT same-engine RAW needs explicit
semaphores (the pipelines do not interlock through SBUF).
"""

from contextlib import ExitStack

import numpy as np

B = 32
IN_CAPS = 2048
IN_DIM = 16
NUM_CAPS = 32
DIM_CAPS = 32
NCORES = 8
JPC = NUM_CAPS // NCORES          # 4 capsules per core
NJL = JPC * DIM_CAPS              # 128 output columns per core
P = 128
NTILES = IN_CAPS // P             # 16
KH = IN_DIM // 2                  # 8 k's per half-row
XH = KH * B                       # 256 packed x floats per half-row (k, b)
WH = KH * NJL                     # 1024 packed w floats per half-row (k, n)
HROW = XH + WH                    # 1280
ROW = 2 * HROW                    # 2560
EPS = 1e-7

_CACHE = {}


def _build():
    import concourse.bass as bass
    from concourse import mybir

    f32 = mybir.dt.float32
    bf16 = mybir.dt.bfloat16
    nc = bass.Bass()
    xw = nc.declare_dram_parameter("xw", [IN_CAPS, ROW], bf16, isOutput=False)
    out = nc.declare_dram_parameter("out", [B, NJL], f32, isOutput=True)

    with ExitStack() as ctx:
        xw_sb = ctx.enter_context(nc.sbuf_tensor([P, NTILES * ROW], bf16))
        sv = ctx.enter_context(nc.sbuf_tensor([B, NJL], f32))
        sq = ctx.enter_context(nc.sbuf_tensor([B, NJL], f32))
        ss = ctx.enter_context(nc.sbuf_tensor([B, JPC], f32))
        rt = ctx.enter_context(nc.sbuf_tensor([B, JPC], f32))
        a1 = ctx.enter_context(nc.sbuf_tensor([B, JPC], f32))
        rc = ctx.enter_context(nc.sbuf_tensor([B, JPC], f32))
        sm = ctx.enter_context(nc.sbuf_tensor([B, JPC], f32))
        fsc = ctx.enter_context(nc.sbuf_tensor([B, JPC], f32))
        epst = ctx.enter_context(nc.sbuf_tensor([B, 1], f32))
        warm = ctx.enter_context(nc.sbuf_tensor([B, 1], f32))
        vout = ctx.enter_context(nc.sbuf_tensor([B, NJL], f32))
        psT = ctx.enter_context(nc.psum_tensor([P, B], f32))

        NDMA = NTILES + 1
        tsem = [ctx.enter_context(nc.semaphore(f"t{t}")) for t in range(NDMA)]
        pe_sem = ctx.enter_context(nc.semaphore("pe"))
        act_sem = ctx.enter_context(nc.semaphore("act"))
        dve_sem = ctx.enter_context(nc.semaphore("dve"))
        odma = ctx.enter_context(nc.semaphore("odma"))
        block = ctx.enter_context(nc.Block())

        @block.sync
        def _(sync):
            for t in range(NTILES - 1):
                sync.dma_start(
                    out=xw_sb[:, t * ROW:(t + 1) * ROW],
                    in_=xw[t * P:(t + 1) * P, :],
                ).then_inc(tsem[t], 16)
            t = NTILES - 1
            for h in range(2):
                sync.dma_start(
                    out=xw_sb[:, t * ROW + h * HROW:t * ROW + (h + 1) * HROW],
                    in_=xw[t * P:(t + 1) * P, h * HROW:(h + 1) * HROW],
                ).then_inc(tsem[t + h], 16)
            sync.wait_ge(dve_sem, 8)
            sync.dma_start(out=out[:, :], in_=vout[:, :]).then_inc(odma, 16)
            sync.wait_ge(odma, 16)

        @block.tensor
        def _(tensor):
            for t in range(NTILES):
                for h in range(2):
                    if h == 0:
                        tensor.wait_ge(tsem[t], 16)
                    elif t == NTILES - 1:
                        tensor.wait_ge(tsem[NTILES], 16)
                    base = t * ROW + h * HROW
                    for k in range(KH):
                        mm = nc.tensor.matmul(
                            psT[:, :],
                            xw_sb[:, base + XH + k * NJL:base + XH + (k + 1) * NJL],
                            xw_sb[:, base + k * B:base + (k + 1) * B],
                            start=(t == 0 and h == 0 and k == 0),
                            stop=(t == NTILES - 1 and h == 1 and k == KH - 1),
                        )
            mm.then_inc(pe_sem, 1)

        @block.vector
        def _(vector):
            nc.vector.memset(epst[:, :], EPS)
            vector.wait_ge(pe_sem, 1)
            # s_T[(j,l), b] -> sv[b, (j,l)] via 4 32x32 stream transposes
            for g in range(JPC):
                tr = nc.vector.transpose(
                    out=sv[:, g * DIM_CAPS:(g + 1) * DIM_CAPS],
                    in_=psT[g * DIM_CAPS:(g + 1) * DIM_CAPS, :],
                )
            tr.then_inc(dve_sem, 1)
            vector.wait_ge(dve_sem, 1)
            nc.vector.tensor_mul(sq[:, :], sv[:, :], sv[:, :]).then_inc(dve_sem, 1)
            vector.wait_ge(dve_sem, 2)
            nc.vector.reduce_sum(
                out=ss[:, :],
                in_=sq[:, :].rearrange("p (g d) -> p g d", g=JPC),
                axis=mybir.AxisListType.X,
            ).then_inc(dve_sem, 1)
            vector.wait_ge(dve_sem, 3)
            nc.vector.tensor_scalar_add(a1[:, :], ss[:, :], 1.0).then_inc(dve_sem, 1)
            # den = (1 + ss) * sqrt(ss + eps); fsc = ss / den
            vector.wait_ge(act_sem, 1)
            vector.wait_ge(dve_sem, 4)
            nc.vector.tensor_mul(sm[:, :], rt[:, :], a1[:, :]).then_inc(dve_sem, 1)
            vector.wait_ge(dve_sem, 5)
            nc.vector.reciprocal(out=rc[:, :], in_=sm[:, :]).then_inc(dve_sem, 1)
            vector.wait_ge(dve_sem, 6)
            nc.vector.tensor_mul(fsc[:, :], ss[:, :], rc[:, :]).then_inc(dve_sem, 1)
            vector.wait_ge(dve_sem, 7)
            nc.vector.tensor_mul(
                vout[:, :].rearrange("p (g d) -> p g d", g=JPC),
                sv[:, :].rearrange("p (g d) -> p g d", g=JPC),
                fsc[:, :, None].broadcast_to((B, JPC, DIM_CAPS)),
            ).then_inc(dve_sem, 1)

        @block.scalar
        def _(scalar):
            # dummy Sqrt pulls the ~1.3us ACT table load off the epilogue
            # critical path (operands are a scratch tile nobody else touches)
            nc.scalar.activation(
                out=warm[:, :], in_=warm[:, :],
                func=mybir.ActivationFunctionType.Sqrt, bias=warm[:, :],
            )
            scalar.wait_ge(dve_sem, 3)
            nc.scalar.activation(
                out=rt[:, :], in_=ss[:, :],
                func=mybir.ActivationFunctionType.Sqrt, bias=epst[:, :],
            ).then_inc(act_sem, 1)

    return nc


def _in_maps(inputs, W):
    import ml_dtypes

    bf = np.dtype(ml_dtypes.bfloat16)
    # x packed k-major per i-row: [i, k, b]
    x_t = np.ascontiguousarray(np.transpose(inputs, (1, 2, 0))).astype(bf)
    maps = []
    for c in range(NCORES):
        # W slice -> [i, k, (j, l)] k-major per i-row
        Wc = np.ascontiguousarray(
            np.transpose(W[:, c * JPC:(c + 1) * JPC], (0, 3, 1, 2))
        ).astype(bf).reshape(IN_CAPS, IN_DIM, NJL)
        xwc = np.empty((IN_CAPS, ROW), dtype=bf)
        for h in range(2):
            o = h * HROW
            xwc[:, o:o + XH] = x_t[:, h * KH:(h + 1) * KH].reshape(IN_CAPS, XH)
            xwc[:, o + XH:o + HROW] = Wc[:, h * KH:(h + 1) * KH].reshape(
                IN_CAPS, WH
            )
        maps.append({"xw": xwc})
    return maps


def kernel(inputs, W):
    from concourse.bass_utils import run_bass_kernel_spmd

    inputs = np.asarray(inputs, dtype=np.float32)
    W = np.asarray(W, dtype=np.float32)
    if "nc" not in _CACHE:
        _CACHE["nc"] = _build()
    res = run_bass_kernel_spmd(_CACHE["nc"], _in_maps(inputs, W), list(range(NCORES)))
    return np.concatenate(
        [res.results[c]["out"].reshape(B, 1, JPC, DIM_CAPS) for c in range(NCORES)],
        axis=2,
    )


# revision 16
# speedup vs baseline: 1.0954x; 1.0954x over previous
"""CapsLayer kernel v7: j-sharded, W-stationary mixed bf16/fp8 contraction.

Math: the reference's routing loop is dead (softmax over a size-1 axis is
identically 1), so the output is
    s[b, j, l] = sum_{i,k} W[i, j, l, k] * inputs[b, i, k]
    vj = squash(s, axis=l)  ->  [B, 1, NUM_CAPS, DIM_CAPS]

Sharding: W splits over NUM_CAPS j (4 capsules / 128 output columns per
core); inputs are replicated.  No collectives (an 8-core ncfw
ReduceScatter measures ~42 us of fixed latency).

Precision: the DMA stream is the wall (the fp32 version already ran at
the ~360 B/ns bus limit), so bytes are the whole game: x streams bf16,
W streams bf16 for even k and fp8e4m3 for odd k (8.3 MB/core vs 21.2
fp32).  PSUM accumulation and the squash run in fp32.  Measured rel err
1.7e-2 vs the 2e-2 gate (bf16-only was 2.1e-3; fp8-ing half of W scales
the fp8-only 2.4e-2 error by sqrt(1/2)).

PE orientation: W[128i, 128(j,l)] is the STATIONARY operand — full 128
columns enables fast-weight-load — and x[128i, 32b] streams as moving
data.  Two interleaved accumulation chains (even k -> psA bf16 weights,
odd k fp8) accumulate one PSUM chain.  The transposed result s_T[(j,l), b]
costs 4 DVE 32x32 stream-transposes in the epilogue.

DMA: 17 transfers — one per 128-row i-tile, last tile split in half (k
0-7 / k 8-15) so the final PE chunk starts half a tile sooner.  The
buffer is byte-packed (uint8) per half-row as [x bf16 512B | W-even bf16
1024B | W-odd fp8 512B]; matmul operands are bitcast views.

Raw Bass: this walrus build rejects instructions carrying 2+ sem waits, so
all sync is standalone wait_ge ops.  DVE/ACT same-engine RAW needs explicit
semaphores (the pipelines do not interlock through SBUF).
"""

from contextlib import ExitStack

import numpy as np

B = 32
IN_CAPS = 2048
IN_DIM = 16
NUM_CAPS = 32
DIM_CAPS = 32
NCORES = 8
JPC = NUM_CAPS // NCORES          # 4 capsules per core
NJL = JPC * DIM_CAPS              # 128 output columns per core
P = 128
NTILES = IN_CAPS // P             # 16
KH = IN_DIM // 2                  # 8 k's per half-row
XB = KH * B * 2                   # 512 B of x per half-row (k-major, bf16)
WEB = (KH // 2) * NJL * 2         # 1024 B of even-k W per half-row (bf16)
WOB = (KH // 2) * NJL             # 512 B of odd-k W per half-row (fp8)
HB = XB + WEB + WOB               # 2048 B per half-row
RB = 2 * HB                       # 4096 B per row
EPS = 1e-7

_CACHE = {}


def _build():
    import concourse.bass as bass
    from concourse import mybir

    f32 = mybir.dt.float32
    bf16 = mybir.dt.bfloat16
    f8 = mybir.dt.float8e4
    u8 = mybir.dt.uint8
    nc = bass.Bass()
    xw = nc.declare_dram_parameter("xw", [IN_CAPS, RB], u8, isOutput=False)
    out = nc.declare_dram_parameter("out", [B, NJL], f32, isOutput=True)

    with ExitStack() as ctx:
        xw_sb = ctx.enter_context(nc.sbuf_tensor([P, NTILES * RB], u8))
        sv = ctx.enter_context(nc.sbuf_tensor([B, NJL], f32))
        sq = ctx.enter_context(nc.sbuf_tensor([B, NJL], f32))
        ss = ctx.enter_context(nc.sbuf_tensor([B, JPC], f32))
        rt = ctx.enter_context(nc.sbuf_tensor([B, JPC], f32))
        a1 = ctx.enter_context(nc.sbuf_tensor([B, JPC], f32))
        rc = ctx.enter_context(nc.sbuf_tensor([B, JPC], f32))
        sm = ctx.enter_context(nc.sbuf_tensor([B, JPC], f32))
        fsc = ctx.enter_context(nc.sbuf_tensor([B, JPC], f32))
        epst = ctx.enter_context(nc.sbuf_tensor([B, 1], f32))
        warm = ctx.enter_context(nc.sbuf_tensor([B, 1], f32))
        vout = ctx.enter_context(nc.sbuf_tensor([B, NJL], f32))
        psT = ctx.enter_context(nc.psum_tensor([P, B], f32))

        NDMA = NTILES + 1
        tsem = [ctx.enter_context(nc.semaphore(f"t{t}")) for t in range(NDMA)]
        pe_sem = ctx.enter_context(nc.semaphore("pe"))
        act_sem = ctx.enter_context(nc.semaphore("act"))
        dve_sem = ctx.enter_context(nc.semaphore("dve"))
        odma = ctx.enter_context(nc.semaphore("odma"))
        block = ctx.enter_context(nc.Block())

        @block.sync
        def _(sync):
            for t in range(NTILES - 1):
                sync.dma_start(
                    out=xw_sb[:, t * RB:(t + 1) * RB],
                    in_=xw[t * P:(t + 1) * P, :],
                ).then_inc(tsem[t], 16)
            t = NTILES - 1
            for h in range(2):
                sync.dma_start(
                    out=xw_sb[:, t * RB + h * HB:t * RB + (h + 1) * HB],
                    in_=xw[t * P:(t + 1) * P, h * HB:(h + 1) * HB],
                ).then_inc(tsem[t + h], 16)
            sync.wait_ge(dve_sem, 8)
            sync.dma_start(out=out[:, :], in_=vout[:, :]).then_inc(odma, 16)
            sync.wait_ge(odma, 16)

        @block.tensor
        def _(tensor):
            last = NTILES * IN_DIM - 1
            n = 0
            for t in range(NTILES):
                for h in range(2):
                    if h == 0:
                        tensor.wait_ge(tsem[t], 16)
                    elif t == NTILES - 1:
                        tensor.wait_ge(tsem[NTILES], 16)
                    base = t * RB + h * HB
                    for a in range(KH // 2):
                        we = xw_sb[:, base + XB + a * 256:base + XB + (a + 1) * 256]
                        xe = xw_sb[:, base + 2 * a * 64:base + (2 * a + 1) * 64]
                        mm = nc.tensor.matmul(
                            psT[:, :],
                            we.bitcast(bf16),
                            xe.bitcast(bf16),
                            start=(n == 0),
                            stop=False,
                        )
                        n += 1
                        wo = xw_sb[:, base + XB + WEB + a * 128:
                                   base + XB + WEB + (a + 1) * 128]
                        xo = xw_sb[:, base + (2 * a + 1) * 64:
                                   base + (2 * a + 2) * 64]
                        mm = nc.tensor.matmul(
                            psT[:, :],
                            wo.bitcast(f8),
                            xo.bitcast(bf16),
                            start=False,
                            stop=(n == last),
                        )
                        n += 1
            mm.then_inc(pe_sem, 1)

        @block.vector
        def _(vector):
            nc.vector.memset(epst[:, :], EPS)
            vector.wait_ge(pe_sem, 1)
            # s_T[(j,l), b] -> sv[b, (j,l)] via 4 32x32 stream transposes
            for g in range(JPC):
                tr = nc.vector.transpose(
                    out=sv[:, g * DIM_CAPS:(g + 1) * DIM_CAPS],
                    in_=psT[g * DIM_CAPS:(g + 1) * DIM_CAPS, :],
                )
            tr.then_inc(dve_sem, 1)
            vector.wait_ge(dve_sem, 1)
            nc.vector.tensor_mul(sq[:, :], sv[:, :], sv[:, :]).then_inc(dve_sem, 1)
            vector.wait_ge(dve_sem, 2)
            nc.vector.reduce_sum(
                out=ss[:, :],
                in_=sq[:, :].rearrange("p (g d) -> p g d", g=JPC),
                axis=mybir.AxisListType.X,
            ).then_inc(dve_sem, 1)
            vector.wait_ge(dve_sem, 3)
            nc.vector.tensor_scalar_add(a1[:, :], ss[:, :], 1.0).then_inc(dve_sem, 1)
            # den = (1 + ss) * sqrt(ss + eps); fsc = ss / den
            vector.wait_ge(act_sem, 1)
            vector.wait_ge(dve_sem, 4)
            nc.vector.tensor_mul(sm[:, :], rt[:, :], a1[:, :]).then_inc(dve_sem, 1)
            vector.wait_ge(dve_sem, 5)
            nc.vector.reciprocal(out=rc[:, :], in_=sm[:, :]).then_inc(dve_sem, 1)
            vector.wait_ge(dve_sem, 6)
            nc.vector.tensor_mul(fsc[:, :], ss[:, :], rc[:, :]).then_inc(dve_sem, 1)
            vector.wait_ge(dve_sem, 7)
            nc.vector.tensor_mul(
                vout[:, :].rearrange("p (g d) -> p g d", g=JPC),
                sv[:, :].rearrange("p (g d) -> p g d", g=JPC),
                fsc[:, :, None].broadcast_to((B, JPC, DIM_CAPS)),
            ).then_inc(dve_sem, 1)

        @block.scalar
        def _(scalar):
            # dummy Sqrt pulls the ~1.3us ACT table load off the epilogue
            # critical path (operands are a scratch tile nobody else touches)
            nc.scalar.activation(
                out=warm[:, :], in_=warm[:, :],
                func=mybir.ActivationFunctionType.Sqrt, bias=warm[:, :],
            )
            scalar.wait_ge(dve_sem, 3)
            nc.scalar.activation(
                out=rt[:, :], in_=ss[:, :],
                func=mybir.ActivationFunctionType.Sqrt, bias=epst[:, :],
            ).then_inc(act_sem, 1)

    return nc


def _in_maps(inputs, W):
    import ml_dtypes

    bf = np.dtype(ml_dtypes.bfloat16)
    f8 = np.dtype(ml_dtypes.float8_e4m3)
    # x packed k-major per i-row: [i, k, b]
    x_t = np.ascontiguousarray(np.transpose(inputs, (1, 2, 0))).astype(bf)
    maps = []
    for c in range(NCORES):
        # W slice -> [i, k, (j, l)] k-major per i-row
        Wc = np.ascontiguousarray(
            np.transpose(W[:, c * JPC:(c + 1) * JPC], (0, 3, 1, 2))
        ).reshape(IN_CAPS, IN_DIM, NJL)
        xwc = np.empty((IN_CAPS, RB), dtype=np.uint8)
        for h in range(2):
            o = h * HB
            xh = np.ascontiguousarray(
                x_t[:, h * KH:(h + 1) * KH]
            ).reshape(IN_CAPS, KH * B)
            xwc[:, o:o + XB] = xh.view(np.uint8)
            we = np.ascontiguousarray(
                Wc[:, h * KH:(h + 1) * KH:2]
            ).astype(bf).reshape(IN_CAPS, WEB // 2)
            xwc[:, o + XB:o + XB + WEB] = we.view(np.uint8)
            wo = np.ascontiguousarray(
                Wc[:, h * KH + 1:(h + 1) * KH:2]
            ).astype(f8).reshape(IN_CAPS, WOB)
            xwc[:, o + XB + WEB:o + HB] = wo.view(np.uint8)
        maps.append({"xw": xwc})
    return maps


def kernel(inputs, W):
    from concourse.bass_utils import run_bass_kernel_spmd

    inputs = np.asarray(inputs, dtype=np.float32)
    W = np.asarray(W, dtype=np.float32)
    if "nc" not in _CACHE:
        _CACHE["nc"] = _build()
    res = run_bass_kernel_spmd(_CACHE["nc"], _in_maps(inputs, W), list(range(NCORES)))
    return np.concatenate(
        [res.results[c]["out"].reshape(B, 1, JPC, DIM_CAPS) for c in range(NCORES)],
        axis=2,
    )


# revision 17
# speedup vs baseline: 1.1512x; 1.0510x over previous
"""CapsLayer kernel v8: j-sharded, W-stationary mixed bf16/fp8 contraction.

Math: the reference's routing loop is dead (softmax over a size-1 axis is
identically 1), so the output is
    s[b, j, l] = sum_{i,k} W[i, j, l, k] * inputs[b, i, k]
    vj = squash(s, axis=l)  ->  [B, 1, NUM_CAPS, DIM_CAPS]

Sharding: W splits over NUM_CAPS j (4 capsules / 128 output columns per
core); inputs are replicated.  No collectives (an 8-core ncfw
ReduceScatter measures ~42 us of fixed latency).

Precision: the DMA stream is the wall (the fp32 version already ran at
the ~360 B/ns bus limit), so bytes are the whole game: x streams bf16,
W streams bf16 for even k and fp8e4m3 for odd k (8.3 MB/core vs 21.2
fp32).  PSUM accumulation and the squash run in fp32.  Measured rel err
1.72e-2 vs the 2e-2 gate, deterministic: it comes entirely from the
host-side quantization in _in_maps, and hardware accumulation matched
the numpy simulation of it exactly (bf16-only variant: 2.1e-3).

PE orientation: W[128i, 128(j,l)] is the STATIONARY operand — full 128
columns enables fast-weight-load — and x[128i, 32b] streams as moving
data.  All 256 matmuls (even k bf16, odd k fp8 stationaries) accumulate
one PSUM chain s_T[(j,l), b]; the transposed result costs 4 DVE 32x32
stream-transposes in the epilogue (cheaper than the v4-era identity-
matmul merge + PSUM copy, and the moving bus carries 4x fewer bytes).

DMA: 18 transfers — one per 128-row i-tile, with the last tile split
half/quarter/quarter so the final PE chunk starts a quarter-tile after
the stream ends.  Rows are quarter-packed [x bf16 256B | W-even bf16
512B | W-odd fp8 256B] x4 so every piece is one contiguous >=1 KB
descriptor per partition, plus a 64 B pad to keep the DRAM row stride
off a 4 KiB boundary; matmul operands are bitcast views into the byte
buffer.

Raw Bass: this walrus build rejects instructions carrying 2+ sem waits, so
all sync is standalone wait_ge ops.  DVE/ACT same-engine RAW needs explicit
semaphores (the pipelines do not interlock through SBUF).
"""

from contextlib import ExitStack

import numpy as np

B = 32
IN_CAPS = 2048
IN_DIM = 16
NUM_CAPS = 32
DIM_CAPS = 32
NCORES = 8
JPC = NUM_CAPS // NCORES          # 4 capsules per core
NJL = JPC * DIM_CAPS              # 128 output columns per core
P = 128
NTILES = IN_CAPS // P             # 16
KQ = IN_DIM // 4                  # 4 k's per quarter-row
XB = KQ * B * 2                   # 256 B of x per quarter-row (k-major, bf16)
WEB = (KQ // 2) * NJL * 2         # 512 B of even-k W per quarter-row (bf16)
WOB = (KQ // 2) * NJL             # 256 B of odd-k W per quarter-row (fp8)
QB = XB + WEB + WOB               # 1024 B per quarter-row
PAD = 64                          # keep DRAM row stride off 4 KiB
RB = 4 * QB + PAD                 # 4160 B per row
EPS = 1e-7

_CACHE = {}


def _build():
    import concourse.bass as bass
    from concourse import mybir

    f32 = mybir.dt.float32
    bf16 = mybir.dt.bfloat16
    f8 = mybir.dt.float8e4
    u8 = mybir.dt.uint8
    nc = bass.Bass()
    xw = nc.declare_dram_parameter("xw", [IN_CAPS, RB], u8, isOutput=False)
    out = nc.declare_dram_parameter("out", [B, NJL], f32, isOutput=True)

    with ExitStack() as ctx:
        xw_sb = ctx.enter_context(nc.sbuf_tensor([P, NTILES * RB], u8))
        sv = ctx.enter_context(nc.sbuf_tensor([B, NJL], f32))
        sq = ctx.enter_context(nc.sbuf_tensor([B, NJL], f32))
        ss = ctx.enter_context(nc.sbuf_tensor([B, JPC], f32))
        rt = ctx.enter_context(nc.sbuf_tensor([B, JPC], f32))
        rc = ctx.enter_context(nc.sbuf_tensor([B, JPC], f32))
        sm = ctx.enter_context(nc.sbuf_tensor([B, JPC], f32))
        fsc = ctx.enter_context(nc.sbuf_tensor([B, JPC], f32))
        epst = ctx.enter_context(nc.sbuf_tensor([B, 1], f32))
        warm = ctx.enter_context(nc.sbuf_tensor([B, 1], f32))
        vout = ctx.enter_context(nc.sbuf_tensor([B, NJL], f32))
        psT = ctx.enter_context(nc.psum_tensor([P, B], f32))

        NDMA = NTILES + 2         # 15 whole tiles + half + quarter + quarter
        tsem = [ctx.enter_context(nc.semaphore(f"t{t}")) for t in range(NDMA)]
        pe_sem = ctx.enter_context(nc.semaphore("pe"))
        act_sem = ctx.enter_context(nc.semaphore("act"))
        dve_sem = ctx.enter_context(nc.semaphore("dve"))
        odma = ctx.enter_context(nc.semaphore("odma"))
        block = ctx.enter_context(nc.Block())

        lt = NTILES - 1
        # last-tile pieces as (start_byte, end_byte) within the row
        pieces = [(0, 2 * QB), (2 * QB, 3 * QB), (3 * QB, RB)]

        @block.sync
        def _(sync):
            for t in range(lt):
                sync.dma_start(
                    out=xw_sb[:, t * RB:(t + 1) * RB],
                    in_=xw[t * P:(t + 1) * P, :],
                ).then_inc(tsem[t], 16)
            for p, (lo, hi) in enumerate(pieces):
                sync.dma_start(
                    out=xw_sb[:, lt * RB + lo:lt * RB + hi],
                    in_=xw[lt * P:(lt + 1) * P, lo:hi],
                ).then_inc(tsem[lt + p], 16)
            sync.wait_ge(dve_sem, 7)
            sync.dma_start(
                out=out[:, :NJL // 2], in_=vout[:, :NJL // 2]
            ).then_inc(odma, 16)
            sync.wait_ge(dve_sem, 8)
            sync.dma_start(
                out=out[:, NJL // 2:], in_=vout[:, NJL // 2:]
            ).then_inc(odma, 16)
            sync.wait_ge(odma, 32)

        @block.tensor
        def _(tensor):
            last = NTILES * IN_DIM - 1
            n = 0
            for t in range(NTILES):
                for q in range(4):
                    if t < lt:
                        if q == 0:
                            tensor.wait_ge(tsem[t], 16)
                    elif q == 0:
                        tensor.wait_ge(tsem[lt], 16)
                    elif q >= 2:
                        tensor.wait_ge(tsem[lt + q - 1], 16)
                    base = t * RB + q * QB
                    for a in range(KQ // 2):
                        we = xw_sb[:, base + XB + a * 256:base + XB + (a + 1) * 256]
                        xe = xw_sb[:, base + 2 * a * 64:base + (2 * a + 1) * 64]
                        mm = nc.tensor.matmul(
                            psT[:, :],
                            we.bitcast(bf16),
                            xe.bitcast(bf16),
                            start=(n == 0),
                            stop=False,
                        )
                        n += 1
                        wo = xw_sb[:, base + XB + WEB + a * 128:
                                   base + XB + WEB + (a + 1) * 128]
                        xo = xw_sb[:, base + (2 * a + 1) * 64:
                                   base + (2 * a + 2) * 64]
                        mm = nc.tensor.matmul(
                            psT[:, :],
                            wo.bitcast(f8),
                            xo.bitcast(bf16),
                            start=False,
                            stop=(n == last),
                        )
                        n += 1
            mm.then_inc(pe_sem, 1)

        @block.vector
        def _(vector):
            nc.vector.memset(epst[:, :], EPS)
            vector.wait_ge(pe_sem, 1)
            # s_T[(j,l), b] -> sv[b, (j,l)] via 4 32x32 stream transposes
            for g in range(JPC):
                tr = nc.vector.transpose(
                    out=sv[:, g * DIM_CAPS:(g + 1) * DIM_CAPS],
                    in_=psT[g * DIM_CAPS:(g + 1) * DIM_CAPS, :],
                )
            tr.then_inc(dve_sem, 1)
            vector.wait_ge(dve_sem, 1)
            nc.vector.tensor_mul(sq[:, :], sv[:, :], sv[:, :]).then_inc(dve_sem, 1)
            vector.wait_ge(dve_sem, 2)
            nc.vector.reduce_sum(
                out=ss[:, :],
                in_=sq[:, :].rearrange("p (g d) -> p g d", g=JPC),
                axis=mybir.AxisListType.X,
            ).then_inc(dve_sem, 1)
            # den = (1 + ss) * sqrt(ss + eps); fsc = ss / den
            vector.wait_ge(act_sem, 1)
            vector.wait_ge(dve_sem, 3)
            nc.vector.scalar_tensor_tensor(
                out=sm[:, :], in0=ss[:, :], scalar=1.0, in1=rt[:, :],
                op0=mybir.AluOpType.add, op1=mybir.AluOpType.mult,
            ).then_inc(dve_sem, 1)
            vector.wait_ge(dve_sem, 4)
            nc.vector.reciprocal(out=rc[:, :], in_=sm[:, :]).then_inc(dve_sem, 1)
            vector.wait_ge(dve_sem, 5)
            nc.vector.tensor_mul(fsc[:, :], ss[:, :], rc[:, :]).then_inc(dve_sem, 1)
            vector.wait_ge(dve_sem, 6)
            half = JPC // 2
            nc.vector.tensor_mul(
                vout[:, :NJL // 2].rearrange("p (g d) -> p g d", g=half),
                sv[:, :NJL // 2].rearrange("p (g d) -> p g d", g=half),
                fsc[:, :half, None].broadcast_to((B, half, DIM_CAPS)),
            ).then_inc(dve_sem, 1)
            vector.wait_ge(dve_sem, 7)
            nc.vector.tensor_mul(
                vout[:, NJL // 2:].rearrange("p (g d) -> p g d", g=half),
                sv[:, NJL // 2:].rearrange("p (g d) -> p g d", g=half),
                fsc[:, half:, None].broadcast_to((B, half, DIM_CAPS)),
            ).then_inc(dve_sem, 1)

        @block.scalar
        def _(scalar):
            # dummy Sqrt pulls the ~1.3us ACT table load off the epilogue
            # critical path (operands are a scratch tile nobody else touches)
            nc.scalar.activation(
                out=warm[:, :], in_=warm[:, :],
                func=mybir.ActivationFunctionType.Sqrt, bias=warm[:, :],
            )
            scalar.wait_ge(dve_sem, 3)
            nc.scalar.activation(
                out=rt[:, :], in_=ss[:, :],
                func=mybir.ActivationFunctionType.Sqrt, bias=epst[:, :],
            ).then_inc(act_sem, 1)

    return nc


def _in_maps(inputs, W):
    import ml_dtypes

    bf = np.dtype(ml_dtypes.bfloat16)
    f8 = np.dtype(ml_dtypes.float8_e4m3)
    # x packed k-major per i-row: [i, k, b]
    x_t = np.ascontiguousarray(np.transpose(inputs, (1, 2, 0))).astype(bf)
    maps = []
    for c in range(NCORES):
        # W slice -> [i, k, (j, l)] k-major per i-row
        Wc = np.ascontiguousarray(
            np.transpose(W[:, c * JPC:(c + 1) * JPC], (0, 3, 1, 2))
        ).reshape(IN_CAPS, IN_DIM, NJL)
        xwc = np.zeros((IN_CAPS, RB), dtype=np.uint8)
        for q in range(4):
            o = q * QB
            xq = np.ascontiguousarray(
                x_t[:, q * KQ:(q + 1) * KQ]
            ).reshape(IN_CAPS, KQ * B)
            xwc[:, o:o + XB] = xq.view(np.uint8)
            we = np.ascontiguousarray(
                Wc[:, q * KQ:(q + 1) * KQ:2]
            ).astype(bf).reshape(IN_CAPS, WEB // 2)
            xwc[:, o + XB:o + XB + WEB] = we.view(np.uint8)
            wo = np.ascontiguousarray(
                Wc[:, q * KQ + 1:(q + 1) * KQ:2]
            ).astype(f8).reshape(IN_CAPS, WOB)
            xwc[:, o + XB + WEB:o + QB] = wo.view(np.uint8)
        maps.append({"xw": xwc})
    return maps


def kernel(inputs, W):
    from concourse.bass_utils import run_bass_kernel_spmd

    inputs = np.asarray(inputs, dtype=np.float32)
    W = np.asarray(W, dtype=np.float32)
    if "nc" not in _CACHE:
        _CACHE["nc"] = _build()
    res = run_bass_kernel_spmd(_CACHE["nc"], _in_maps(inputs, W), list(range(NCORES)))
    return np.concatenate(
        [res.results[c]["out"].reshape(B, 1, JPC, DIM_CAPS) for c in range(NCORES)],
        axis=2,
    )
